# revision 1
# baseline (speedup 1.0000x reference)
"""Graph-transformer layer (GTLayer) on 8 Trainium2 NeuronCores.

Strategy (node-parallel, host-side edge binning as the sharding step):
  - Sort edges by destination node (row). Core c owns nodes
    [c*6250, (c+1)*6250) and receives exactly the edges pointing at its
    nodes, binned into 49 blocks of 128 destination nodes, each padded to
    T_B tiles of 128 edge slots (pad slots have local-id -1).
  - On device, each core computes k/v projection tables for ALL nodes
    (replicated work, no collectives) and a q table for its local nodes,
    then for each edge tile gathers q[row], k[col], v[col] via indirect
    DMA, computes per-head attention weights, and scatter-adds
    [weighted-v | exp-weight] into a per-block PSUM accumulator via a
    one-hot selection matmul (edges x local-node one-hot as lhsT).
  - Finalize per block: divide by (exp-sum + 1e-8), add residual,
    LayerNorm(eps=1e-6), write the block's 128 output rows.
  - Host concatenates the 8 per-core [6250, 128] outputs.

All cores run one identical program; per-core behavior differs only
through input data (binned index arrays + local embed slices).
"""

import numpy as np

import concourse.bass as bass
import concourse.bacc as bacc
import concourse.tile as tile
from concourse import mybir
from concourse.bass_utils import run_bass_kernel_spmd
from concourse.masks import make_identity

N = 50000
E = 800000
D = 128
H = 8
HD = 16
NCORES = 8
NPC = N // NCORES  # 6250 nodes per core
NB = (NPC + 127) // 128  # 49 blocks of 128 nodes per core
NBP = NB * 128  # 6272 padded local nodes
NPAD = ((N + 127) // 128) * 128  # 50048 padded table rows
NT = NPAD // 128  # 391 table blocks

F32 = mybir.dt.float32
I32 = mybir.dt.int32


def _bcast_inner(ap2d: bass.AP, k: int) -> bass.AP:
    """View a [P, m] AP as [P, m, k] with the inner dim broadcast (step 0)."""
    return bass.AP(tensor=ap2d.tensor, offset=ap2d.offset, ap=[*ap2d.ap, [0, k]])


def _head_view(ap2d: bass.AP) -> bass.AP:
    """View a [P, D] AP as [P, H, HD]."""
    return ap2d.rearrange("p (h x) -> p h x", h=H)


def build_program(t_b: int) -> bass.Bass:
    nc = bacc.Bacc(None, num_swdge_queues=4)

    embeds = nc.dram_tensor("embeds", [N, D], F32, kind="ExternalInput")
    emb_local = nc.dram_tensor("emb_local", [NBP, D], F32, kind="ExternalInput")
    qT = nc.dram_tensor("qT", [D, D], F32, kind="ExternalInput")
    kT = nc.dram_tensor("kT", [D, D], F32, kind="ExternalInput")
    vT = nc.dram_tensor("vT", [D, D], F32, kind="ExternalInput")
    lnsc = nc.dram_tensor("lnsc", [D], F32, kind="ExternalInput")
    lnb = nc.dram_tensor("lnb", [D], F32, kind="ExternalInput")
    lloc_d = nc.dram_tensor("lloc", [NB * 128, t_b], F32, kind="ExternalInput")
    qidx_d = nc.dram_tensor("qidx", [NB * 128, t_b], I32, kind="ExternalInput")
    cidx_d = nc.dram_tensor("cidx", [NB * 128, t_b], I32, kind="ExternalInput")

    kNodes = nc.dram_tensor("kNodes", [NPAD, D], F32)
    vNodes = nc.dram_tensor("vNodes", [NPAD, D], F32)
    qNodes = nc.dram_tensor("qNodes", [NBP, D], F32)

    out_d = nc.dram_tensor("out", [NBP, D], F32, kind="ExternalOutput")

    with tile.TileContext(nc) as tc:
        with tc.tile_pool(name="singles", bufs=1) as singles:
            # ---- one-time constants ----
            ident_g = singles.tile([128, 128], F32)
            make_identity(nc, ident_g)
            ident = singles.tile([128, 128], F32)
            nc.vector.tensor_copy(ident[:], ident_g[:])

            iota_i = singles.tile([128, 128], I32)
            nc.gpsimd.iota(iota_i[:], pattern=[[1, 128]], base=0, channel_multiplier=0)
            iota_f = singles.tile([128, 128], F32)
            nc.vector.tensor_copy(iota_f[:], iota_i[:])

            lnsc_t = singles.tile([128, 128], F32)
            nc.sync.dma_start(
                out=lnsc_t[:],
                in_=bass.AP(tensor=lnsc, offset=0, ap=[[0, 128], [1, 128]]),
            )
            lnb_t = singles.tile([128, 128], F32)
            nc.sync.dma_start(
                out=lnb_t[:],
                in_=bass.AP(tensor=lnb, offset=0, ap=[[0, 128], [1, 128]]),
            )
            eps_t = singles.tile([128, 1], F32)
            nc.vector.memset(eps_t[:], 1e-6)

            qT_t = singles.tile([128, 128], F32)
            nc.sync.dma_start(qT_t[:], qT[:])
            kT_t = singles.tile([128, 128], F32)
            nc.sync.dma_start(kT_t[:], kT[:])
            vT_t = singles.tile([128, 128], F32)
            nc.sync.dma_start(vT_t[:], vT[:])

            # ---- phase A: projection tables ----
            ctx_a = tc.tile_pool(name="tA", bufs=3)
            tA = ctx_a.__enter__()
            ctx_aps = tc.tile_pool(name="psA", bufs=2, space="PSUM")
            psA = ctx_aps.__enter__()
            for b in range(NT):
                sz = min(128, N - b * 128)
                emb_t = tA.tile([128, 128], F32)
                if sz < 128:
                    nc.vector.memset(emb_t[:], 0.0)
                nc.sync.dma_start(emb_t[:sz, :], embeds[b * 128 : b * 128 + sz, :])
                tp = psA.tile([128, 128], F32)
                nc.tensor.transpose(out=tp[:], in_=emb_t[:], identity=ident[:])
                embT = tA.tile([128, 128], F32)
                nc.scalar.copy(embT[:], tp[:])

                kp = psA.tile([128, 128], F32)
                nc.tensor.matmul(kp[:], lhsT=embT[:], rhs=kT_t[:], start=True, stop=True)
                ks = tA.tile([128, 128], F32)
                nc.vector.tensor_copy(ks[:], kp[:])
                nc.sync.dma_start(kNodes[b * 128 : (b + 1) * 128, :], ks[:])

                vp = psA.tile([128, 128], F32)
                nc.tensor.matmul(vp[:], lhsT=embT[:], rhs=vT_t[:], start=True, stop=True)
                vs = tA.tile([128, 128], F32)
                nc.vector.tensor_copy(vs[:], vp[:])
                nc.sync.dma_start(vNodes[b * 128 : (b + 1) * 128, :], vs[:])

            for b in range(NB):
                emb_t = tA.tile([128, 128], F32)
                nc.sync.dma_start(emb_t[:], emb_local[b * 128 : (b + 1) * 128, :])
                tp = psA.tile([128, 128], F32)
                nc.tensor.transpose(out=tp[:], in_=emb_t[:], identity=ident[:])
                embT = tA.tile([128, 128], F32)
                nc.scalar.copy(embT[:], tp[:])
                qp = psA.tile([128, 128], F32)
                nc.tensor.matmul(qp[:], lhsT=embT[:], rhs=qT_t[:], start=True, stop=True)
                qs = tA.tile([128, 128], F32)
                nc.vector.tensor_copy(qs[:], qp[:])
                nc.sync.dma_start(qNodes[b * 128 : (b + 1) * 128, :], qs[:])

            ctx_aps.__exit__(None, None, None)
            ctx_a.__exit__(None, None, None)

            # ---- phase B: edge tiles + scatter + finalize ----
            ctx_idx = tc.tile_pool(name="idxp", bufs=3)
            idxp = ctx_idx.__enter__()
            ctx_gat = tc.tile_pool(name="gat", bufs=12)
            gat = ctx_gat.__enter__()
            ctx_work = tc.tile_pool(name="work", bufs=8)
            work = ctx_work.__enter__()
            ctx_acc = tc.tile_pool(name="accps", bufs=4, space="PSUM")
            accps = ctx_acc.__enter__()
            ctx_fin = tc.tile_pool(name="finp", bufs=3)
            finp = ctx_fin.__enter__()
            for n in range(NB):
                r0 = n * 128
                lloc_t = idxp.tile([128, t_b], F32)
                nc.sync.dma_start(lloc_t[:], lloc_d[r0 : r0 + 128, :])
                qix_t = idxp.tile([128, t_b], I32)
                nc.sync.dma_start(qix_t[:], qidx_d[r0 : r0 + 128, :])
                cix_t = idxp.tile([128, t_b], I32)
                nc.sync.dma_start(cix_t[:], cidx_d[r0 : r0 + 128, :])

                acc = accps.tile([128, 136], F32)

                for t in range(t_b):
                    q_t = gat.tile([128, 128], F32)
                    nc.gpsimd.indirect_dma_start(
                        out=q_t[:],
                        out_offset=None,
                        in_=qNodes[:],
                        in_offset=bass.IndirectOffsetOnAxis(
                            ap=qix_t[:, t : t + 1], axis=0
                        ),
                    )
                    k_t = gat.tile([128, 128], F32)
                    nc.gpsimd.indirect_dma_start(
                        out=k_t[:],
                        out_offset=None,
                        in_=kNodes[:],
                        in_offset=bass.IndirectOffsetOnAxis(
                            ap=cix_t[:, t : t + 1], axis=0
                        ),
                    )
                    v_t = gat.tile([128, 128], F32)
                    nc.gpsimd.indirect_dma_start(
                        out=v_t[:],
                        out_offset=None,
                        in_=vNodes[:],
                        in_offset=bass.IndirectOffsetOnAxis(
                            ap=cix_t[:, t : t + 1], axis=0
                        ),
                    )

                    qk = work.tile([128, 128], F32)
                    nc.vector.tensor_tensor(
                        out=qk[:], in0=q_t[:], in1=k_t[:], op=mybir.AluOpType.mult
                    )
                    att = work.tile([128, H], F32)
                    nc.vector.tensor_reduce(
                        out=att[:],
                        in_=_head_view(qk[:]),
                        op=mybir.AluOpType.add,
                        axis=mybir.AxisListType.X,
                    )
                    attc = work.tile([128, H], F32)
                    nc.vector.tensor_scalar(
                        out=attc[:],
                        in0=att[:],
                        scalar1=10.0,
                        scalar2=-10.0,
                        op0=mybir.AluOpType.min,
                        op1=mybir.AluOpType.max,
                    )
                    expw = work.tile([128, H], F32)
                    nc.scalar.activation(
                        out=expw[:], in_=attc[:], func=mybir.ActivationFunctionType.Exp
                    )

                    x_t = work.tile([128, 136], F32)
                    nc.vector.tensor_tensor(
                        out=_head_view(x_t[:, 0:128]),
                        in0=_head_view(v_t[:]),
                        in1=_bcast_inner(expw[:], HD),
                        op=mybir.AluOpType.mult,
                    )
                    nc.gpsimd.tensor_copy(x_t[:, 128:136], expw[:])

                    p_t = work.tile([128, 128], F32)
                    nc.vector.tensor_scalar(
                        out=p_t[:],
                        in0=iota_f[:],
                        scalar1=lloc_t[:, t : t + 1],
                        scalar2=None,
                        op0=mybir.AluOpType.is_equal,
                    )

                    nc.tensor.matmul(
                        acc[:],
                        lhsT=p_t[:],
                        rhs=x_t[:],
                        start=(t == 0),
                        stop=(t == t_b - 1),
                    )

                # finalize block n
                accs = finp.tile([128, 136], F32)
                nc.vector.tensor_copy(accs[:], acc[:])
                dinv = finp.tile([128, H], F32)
                nc.vector.tensor_scalar_add(dinv[:], accs[:, 128:136], 1e-8)
                nc.vector.reciprocal(dinv[:], dinv[:])

                emb_t = finp.tile([128, 128], F32)
                nc.sync.dma_start(emb_t[:], emb_local[r0 : r0 + 128, :])

                res = finp.tile([128, 128], F32)
                nc.vector.tensor_tensor(
                    out=_head_view(res[:]),
                    in0=_head_view(accs[:, 0:128]),
                    in1=_bcast_inner(dinv[:], HD),
                    op=mybir.AluOpType.mult,
                )
                nc.vector.tensor_add(res[:], res[:], emb_t[:])

                stats = finp.tile([128, 6], F32)
                nc.vector.bn_stats(out=stats[:], in_=res[:])
                mv = finp.tile([128, 2], F32)
                nc.vector.bn_aggr(out=mv[:], in_=stats[:])

                sd = finp.tile([128, 1], F32)
                nc.scalar.activation(
                    out=sd[:],
                    in_=mv[:, 1:2],
                    func=mybir.ActivationFunctionType.Sqrt,
                    bias=eps_t[:],
                    scale=1.0,
                )
                nc.vector.reciprocal(sd[:], sd[:])

                xm = finp.tile([128, 128], F32)
                nc.vector.tensor_scalar_sub(xm[:], res[:], mv[:, 0:1])
                y = finp.tile([128, 128], F32)
                nc.vector.scalar_tensor_tensor(
                    out=y[:],
                    in0=xm[:],
                    scalar=sd[:],
                    in1=lnsc_t[:],
                    op0=mybir.AluOpType.mult,
                    op1=mybir.AluOpType.mult,
                )
                nc.vector.tensor_add(y[:], y[:], lnb_t[:])
                nc.sync.dma_start(out_d[r0 : r0 + 128, :], y[:])

            ctx_fin.__exit__(None, None, None)
            ctx_acc.__exit__(None, None, None)
            ctx_work.__exit__(None, None, None)
            ctx_gat.__exit__(None, None, None)
            ctx_idx.__exit__(None, None, None)

    nc.finalize()
    return nc


def _prepare_core_inputs(embeds, edge_index, qTrans, kTrans, vTrans, ln_scale, ln_bias):
    rows = np.asarray(edge_index[0]).astype(np.int64)
    cols = np.asarray(edge_index[1]).astype(np.int64)

    order = np.argsort(rows, kind="stable")
    rs = rows[order]
    cs = cols[order]

    core = rs // NPC
    local = rs - core * NPC
    blk = local >> 7
    lloc = (local & 127).astype(np.float32)
    g = core * NB + blk  # global block id, nondecreasing

    counts = np.bincount(g, minlength=NCORES * NB)
    t_b = max(2, int(np.ceil(counts.max() / 128)))
    cap = t_b * 128

    starts = np.zeros(NCORES * NB, dtype=np.int64)
    np.cumsum(counts[:-1], out=starts[1:])
    pos = np.arange(E, dtype=np.int64) - starts[g]
    slot = g * cap + pos

    nslots = NCORES * NB * cap
    lloc_a = np.full(nslots, -1.0, dtype=np.float32)
    qidx_a = np.zeros(nslots, dtype=np.int32)
    cidx_a = np.zeros(nslots, dtype=np.int32)
    lloc_a[slot] = lloc
    qidx_a[slot] = local.astype(np.int32)
    cidx_a[slot] = cs.astype(np.int32)

    # [ncores*NB, t_b, 128] -> [ncores, NB*128, t_b] so each block's
    # [128, t_b] SBUF tile is one contiguous DMA (partition p = edge lane,
    # column t = tile index).
    def to_tiles(a):
        a = a.reshape(NCORES, NB, t_b, 128).transpose(0, 1, 3, 2)
        return np.ascontiguousarray(a.reshape(NCORES, NB * 128, t_b))

    lloc_a = to_tiles(lloc_a)
    qidx_a = to_tiles(qidx_a)
    cidx_a = to_tiles(cidx_a)

    embeds = np.ascontiguousarray(np.asarray(embeds, dtype=np.float32))
    emb_pad = np.zeros((NCORES, NBP, D), dtype=np.float32)
    emb_pad[:, :NPC, :] = embeds.reshape(NCORES, NPC, D)

    qTrans = np.ascontiguousarray(np.asarray(qTrans, dtype=np.float32))
    kTrans = np.ascontiguousarray(np.asarray(kTrans, dtype=np.float32))
    vTrans = np.ascontiguousarray(np.asarray(vTrans, dtype=np.float32))
    ln_scale = np.ascontiguousarray(np.asarray(ln_scale, dtype=np.float32))
    ln_bias = np.ascontiguousarray(np.asarray(ln_bias, dtype=np.float32))

    in_maps = []
    for c in range(NCORES):
        in_maps.append(
            {
                "embeds": embeds,
                "emb_local": emb_pad[c],
                "qT": qTrans,
                "kT": kTrans,
                "vT": vTrans,
                "lnsc": ln_scale,
                "lnb": ln_bias,
                "lloc": lloc_a[c],
                "qidx": qidx_a[c],
                "cidx": cidx_a[c],
            }
        )
    return in_maps, t_b


_PROGRAM_CACHE: dict[int, bass.Bass] = {}


def kernel(embeds, edge_index, qTrans, kTrans, vTrans, ln_scale, ln_bias, **_):
    in_maps, t_b = _prepare_core_inputs(
        embeds, edge_index, qTrans, kTrans, vTrans, ln_scale, ln_bias
    )
    nc = _PROGRAM_CACHE.get(t_b)
    if nc is None:
        nc = build_program(t_b)
        _PROGRAM_CACHE[t_b] = nc

    res = run_bass_kernel_spmd(nc, in_maps, core_ids=list(range(NCORES)))
    outs = [res.results[c]["out"][:NPC] for c in range(NCORES)]
    return np.concatenate(outs, axis=0)


if __name__ == "__main__":
    rng = np.random.default_rng(0)
    inputs = {
        "embeds": rng.standard_normal((N, D), dtype=np.float32),
        "edge_index": rng.integers(0, N, size=(2, E)).astype(np.int64),
        "qTrans": (rng.standard_normal((D, D), dtype=np.float32) / np.sqrt(D)).astype(
            np.float32
        ),
        "kTrans": (rng.standard_normal((D, D), dtype=np.float32) / np.sqrt(D)).astype(
            np.float32
        ),
        "vTrans": (rng.standard_normal((D, D), dtype=np.float32) / np.sqrt(D)).astype(
            np.float32
        ),
        "ln_scale": np.ones(D, dtype=np.float32),
        "ln_bias": np.zeros(D, dtype=np.float32),
    }
    out = kernel(**inputs)
    print("kernel output", out.shape, out.dtype, np.isfinite(out).all())



# revision 2
# speedup vs baseline: 1.1383x; 1.1383x over previous
"""Graph-transformer layer (GTLayer) on 8 Trainium2 NeuronCores — v2.

Differences from the v1 baseline (per-core, all cores run one program):
  - kv table: one [NPAD, 256] bf16 DRAM table holding [k | v] per node, so
    each edge tile needs ONE indirect gather (v1 needed three: q, k, v).
    Indirect DMA costs ~1us fixed on the Pool engine per instruction, so
    this is the dominant win.
  - q is never gathered: each destination block's q rows live in a
    per-block [128, 128] SBUF tile (q_blk); per-edge q rows are selected
    with a one-hot matmul (lhsT = p2, the [local, edge] one-hot built from
    a partition-broadcast copy of the local ids).
  - phase A batches embed loads 4 blocks per DMA and kv stores 4 blocks
    per DMA (HWDGE fixed cost is per-DMA).
  - one-hots / kv / x_t are bf16: halves DVE and PE time and gather bytes.
    exp-weights and the attention accumulate stay f32.
"""

import numpy as np
import ml_dtypes

import concourse.bass as bass
import concourse.bacc as bacc
import concourse.tile as tile
from concourse import mybir
from concourse.bass_utils import run_bass_kernel_spmd
from concourse.masks import make_identity

N = 50000
E = 800000
D = 128
H = 8
HD = 16
NCORES = 8
NPC = N // NCORES  # 6250 nodes per core
NB = (NPC + 127) // 128  # 49 blocks of 128 dst nodes per core
NBP = NB * 128  # 6272 padded local nodes
NPAD = ((N + 127) // 128) * 128  # 50048 padded table rows
NT = NPAD // 128  # 391 table blocks
GA = 4  # phase-A blocks per load/store group
NG = (NT + GA - 1) // GA  # 98 groups (last has 3 blocks)

F32 = mybir.dt.float32
BF16 = mybir.dt.bfloat16
I32 = mybir.dt.int32

KV = 2 * D  # 256 cols: [k | v]


def _bcast_inner(ap2d: bass.AP, k: int) -> bass.AP:
    """View a [P, m] AP as [P, m, k] with the inner dim broadcast (step 0)."""
    return bass.AP(tensor=ap2d.tensor, offset=ap2d.offset, ap=[*ap2d.ap, [0, k]])


def _head_view(ap2d: bass.AP) -> bass.AP:
    return ap2d.rearrange("p (h x) -> p h x", h=H)


import os
DBG_MODE = os.environ.get("KV2_MODE", "full")


def build_program(t_b: int) -> bass.Bass:
    nc = bacc.Bacc(None, num_swdge_queues=4)

    embeds = nc.dram_tensor("embeds", [N, D], F32, kind="ExternalInput")
    emb_local = nc.dram_tensor("emb_local", [NBP, D], F32, kind="ExternalInput")
    qT = nc.dram_tensor("qT", [D, D], F32, kind="ExternalInput")
    kT = nc.dram_tensor("kT", [D, D], F32, kind="ExternalInput")
    vT = nc.dram_tensor("vT", [D, D], F32, kind="ExternalInput")
    lnsc = nc.dram_tensor("lnsc", [D], F32, kind="ExternalInput")
    lnb = nc.dram_tensor("lnb", [D], F32, kind="ExternalInput")
    lloc_d = nc.dram_tensor("lloc", [NB * 128, t_b], F32, kind="ExternalInput")
    llocr_d = nc.dram_tensor("llocr", [NB, t_b * 128], BF16, kind="ExternalInput")
    cidx_d = nc.dram_tensor("cidx", [NB * 128, t_b], I32, kind="ExternalInput")

    kv_d = nc.dram_tensor("kv", [NPAD, KV], BF16)

    out_d = nc.dram_tensor("out", [NBP, D], F32, kind="ExternalOutput")

    with tile.TileContext(nc) as tc:
        with tc.tile_pool(name="singles", bufs=1) as singles:
            # ---- one-time constants ----
            ident_g = singles.tile([128, 128], F32)
            make_identity(nc, ident_g)
            ident = singles.tile([128, 128], F32)
            nc.vector.tensor_copy(ident[:], ident_g[:])

            iota_i = singles.tile([128, 128], I32)
            nc.gpsimd.iota(iota_i[:], pattern=[[1, 128]], base=0, channel_multiplier=0)
            iota_f = singles.tile([128, 128], BF16)
            nc.vector.tensor_copy(iota_f[:], iota_i[:])

            iotap_i = singles.tile([128, 1], I32)
            nc.gpsimd.iota(iotap_i[:], pattern=[[1, 1]], base=0, channel_multiplier=1)
            iota_p = singles.tile([128, 1], F32)
            nc.vector.tensor_copy(iota_p[:], iotap_i[:])

            lnsc_t = singles.tile([128, 128], F32)
            nc.sync.dma_start(
                out=lnsc_t[:],
                in_=bass.AP(tensor=lnsc, offset=0, ap=[[0, 128], [1, 128]]),
            )
            lnb_t = singles.tile([128, 128], F32)
            nc.sync.dma_start(
                out=lnb_t[:],
                in_=bass.AP(tensor=lnb, offset=0, ap=[[0, 128], [1, 128]]),
            )
            eps_t = singles.tile([128, 1], F32)
            nc.vector.memset(eps_t[:], 1e-6)

            qT_t = singles.tile([128, 128], F32)
            nc.sync.dma_start(qT_t[:], qT[:])
            kT_t = singles.tile([128, 128], F32)
            nc.sync.dma_start(kT_t[:], kT[:])
            vT_t = singles.tile([128, 128], F32)
            nc.sync.dma_start(vT_t[:], vT[:])

            # ---- phase A: kv projection table (groups of GA blocks) ----
            ctx_a = tc.tile_pool(name="tA", bufs=3)
            tA = ctx_a.__enter__()
            ctx_aps = tc.tile_pool(name="psA", bufs=3, space="PSUM")
            psA = ctx_aps.__enter__()
            groups = [(g * GA, GA) for g in range(NT // GA)]
            for b in range((NT // GA) * GA, NT):
                groups.append((b, 1))
            for b0, nb in groups:
                r0 = b0 * 128
                rows = min(128 * nb, N - r0)  # valid embed rows in group
                emb_t = tA.tile([128, 128 * nb], F32)
                if rows < 128 * nb:
                    nc.vector.memset(emb_t[:], 0.0)
                    nc.sync.dma_start(
                        emb_t[:rows, 0:128], embeds[r0 : r0 + rows, :]
                    )
                else:
                    # emb_t[p, j, :] = embeds[r0 + j*128 + p, :]
                    nc.sync.dma_start(
                        emb_t[:].rearrange("p (j c) -> p j c", c=128),
                        bass.AP(
                            tensor=embeds,
                            offset=r0 * D,
                            ap=[[D, 128], [128 * D, nb], [1, D]],
                        ),
                    )
                kv_sb = tA.tile([128, KV * nb], BF16)
                for j in range(nb):
                    tp = psA.tile([128, 128], F32)
                    nc.tensor.transpose(
                        out=tp[:], in_=emb_t[:, j * 128 : (j + 1) * 128], identity=ident[:]
                    )
                    embT = tA.tile([128, 128], F32)
                    nc.scalar.copy(embT[:], tp[:])

                    kv_ps = psA.tile([128, KV], F32)
                    nc.tensor.matmul(
                        kv_ps[:, 0:D], lhsT=embT[:], rhs=kT_t[:], start=True, stop=True
                    )
                    nc.tensor.matmul(
                        kv_ps[:, D:KV], lhsT=embT[:], rhs=vT_t[:], start=True, stop=True
                    )
                    nc.vector.tensor_copy(kv_sb[:, j * KV : (j + 1) * KV], kv_ps[:])
                # kv_d[r0 + j*128 + p, :] = kv_sb[p, j, :]
                nc.sync.dma_start(
                    bass.AP(
                        tensor=kv_d,
                        offset=r0 * KV,
                        ap=[[KV, 128], [128 * KV, nb], [1, KV]],
                    ),
                    kv_sb[:].rearrange("p (j c) -> p j c", c=KV),
                )
            ctx_aps.__exit__(None, None, None)
            ctx_a.__exit__(None, None, None)

            # ---- phase B ----
            run_phase_b = DBG_MODE != "phaseA"
            if not run_phase_b:
                # dump kv rows 0..127 to out[0:128] and stop
                kv_chk = singles.tile([128, KV], BF16)
                nc.gpsimd.indirect_dma_start(
                    out=kv_chk[:], out_offset=None, in_=kv_d[:],
                    in_offset=bass.IndirectOffsetOnAxis(ap=iotap_i[:], axis=0),
                )
                kv_f = singles.tile([128, KV], F32)
                nc.vector.tensor_copy(kv_f[:], kv_chk[:])
                nc.sync.dma_start(out_d[0:128, :], kv_f[:, 0:128])
            ctx_idx = tc.tile_pool(name="idxp", bufs=2)
            idxp = ctx_idx.__enter__()
            ctx_emb = tc.tile_pool(name="embp", bufs=2)
            embp = ctx_emb.__enter__()
            ctx_qps = tc.tile_pool(name="qps", bufs=1, space="PSUM")
            qps = ctx_qps.__enter__()
            ctx_gat = tc.tile_pool(name="gat", bufs=12)
            gat = ctx_gat.__enter__()
            ctx_work = tc.tile_pool(name="work", bufs=10)
            work = ctx_work.__enter__()
            ctx_sel = tc.tile_pool(name="selps", bufs=2, space="PSUM")
            selps = ctx_sel.__enter__()
            ctx_acc = tc.tile_pool(name="accps", bufs=2, space="PSUM")
            accps = ctx_acc.__enter__()
            ctx_fin = tc.tile_pool(name="finp", bufs=3)
            finp = ctx_fin.__enter__()

            for n in range(NB if run_phase_b else 0):
                r0 = n * 128
                cix_t = idxp.tile([128, t_b], I32)
                nc.scalar.dma_start(cix_t[:], cidx_d[r0 : r0 + 128, :])
                lloc_t = idxp.tile([128, t_b], F32)
                nc.scalar.dma_start(lloc_t[:], lloc_d[r0 : r0 + 128, :])
                llocr_t = idxp.tile([128, t_b * 128], BF16)
                nc.sync.dma_start(
                    out=llocr_t[:],
                    in_=bass.AP(
                        tensor=llocr_d, offset=n * t_b * 128, ap=[[0, 128], [1, t_b * 128]]
                    ),
                )

                # q_blk for this block's 128 local nodes
                emb_t = embp.tile([128, 128], F32)
                nc.sync.dma_start(emb_t[:], emb_local[r0 : r0 + 128, :])
                tp = qps.tile([128, 128], F32)
                nc.tensor.transpose(out=tp[:], in_=emb_t[:], identity=ident[:])
                embT = embp.tile([128, 128], F32)
                nc.scalar.copy(embT[:], tp[:])
                q_ps = qps.tile([128, 128], F32)
                nc.tensor.matmul(q_ps[:], lhsT=embT[:], rhs=qT_t[:], start=True, stop=True)
                q_blk = embp.tile([128, 128], BF16)
                nc.scalar.copy(q_blk[:], q_ps[:])

                acc = accps.tile([128, 136], F32)

                for t in range(t_b):
                    kv_t = gat.tile([128, KV], BF16)
                    nc.gpsimd.indirect_dma_start(
                        out=kv_t[:],
                        out_offset=None,
                        in_=kv_d[:],
                        in_offset=bass.IndirectOffsetOnAxis(
                            ap=cix_t[:, t : t + 1], axis=0
                        ),
                    )

                    p2 = work.tile([128, 128], BF16)
                    nc.vector.tensor_scalar(
                        out=p2[:],
                        in0=llocr_t[:, t * 128 : (t + 1) * 128],
                        scalar1=iota_p[:],
                        scalar2=None,
                        op0=mybir.AluOpType.is_equal,
                    )
                    q_sel = selps.tile([128, 128], F32)
                    nc.tensor.matmul(
                        q_sel[:], lhsT=p2[:], rhs=q_blk[:], start=True, stop=True
                    )

                    p_t = work.tile([128, 128], BF16)
                    nc.vector.tensor_scalar(
                        out=p_t[:],
                        in0=iota_f[:],
                        scalar1=lloc_t[:, t : t + 1],
                        scalar2=None,
                        op0=mybir.AluOpType.is_equal,
                    )

                    qk = work.tile([128, 128], F32)
                    nc.vector.tensor_tensor(
                        out=qk[:], in0=q_sel[:], in1=kv_t[:, 0:D], op=mybir.AluOpType.mult
                    )
                    att = work.tile([128, H], F32)
                    nc.vector.tensor_reduce(
                        out=att[:],
                        in_=_head_view(qk[:]),
                        op=mybir.AluOpType.add,
                        axis=mybir.AxisListType.X,
                    )
                    attc = work.tile([128, H], F32)
                    nc.vector.tensor_scalar(
                        out=attc[:],
                        in0=att[:],
                        scalar1=10.0,
                        scalar2=-10.0,
                        op0=mybir.AluOpType.min,
                        op1=mybir.AluOpType.max,
                    )
                    expw = work.tile([128, H], F32)
                    nc.scalar.activation(
                        out=expw[:], in_=attc[:], func=mybir.ActivationFunctionType.Exp
                    )

                    x_t = work.tile([128, 136], BF16)
                    nc.scalar.copy(x_t[:, 128:136], expw[:])
                    nc.vector.tensor_tensor(
                        out=_head_view(x_t[:, 0:128]),
                        in0=_head_view(kv_t[:, D:KV]),
                        in1=_bcast_inner(x_t[:, 128:136], HD),
                        op=mybir.AluOpType.mult,
                    )

                    nc.tensor.matmul(
                        acc[:],
                        lhsT=p_t[:],
                        rhs=x_t[:],
                        start=(t == 0),
                        stop=(t == t_b - 1),
                    )

                # finalize block n
                accs = finp.tile([128, 136], F32)
                nc.vector.tensor_copy(accs[:], acc[:])
                dinv = finp.tile([128, H], F32)
                nc.vector.tensor_scalar_add(dinv[:], accs[:, 128:136], 1e-8)
                nc.vector.reciprocal(dinv[:], dinv[:])

                res = finp.tile([128, 128], F32)
                nc.vector.tensor_tensor(
                    out=_head_view(res[:]),
                    in0=_head_view(accs[:, 0:128]),
                    in1=_bcast_inner(dinv[:], HD),
                    op=mybir.AluOpType.mult,
                )
                nc.vector.tensor_add(res[:], res[:], emb_t[:])

                stats = finp.tile([128, 6], F32)
                nc.vector.bn_stats(out=stats[:], in_=res[:])
                mv = finp.tile([128, 2], F32)
                nc.vector.bn_aggr(out=mv[:], in_=stats[:])

                sd = finp.tile([128, 1], F32)
                nc.scalar.activation(
                    out=sd[:],
                    in_=mv[:, 1:2],
                    func=mybir.ActivationFunctionType.Sqrt,
                    bias=eps_t[:],
                    scale=1.0,
                )
                nc.vector.reciprocal(sd[:], sd[:])

                xm = finp.tile([128, 128], F32)
                nc.vector.tensor_scalar_sub(xm[:], res[:], mv[:, 0:1])
                y = finp.tile([128, 128], F32)
                nc.vector.scalar_tensor_tensor(
                    out=y[:],
                    in0=xm[:],
                    scalar=sd[:],
                    in1=lnsc_t[:],
                    op0=mybir.AluOpType.mult,
                    op1=mybir.AluOpType.mult,
                )
                nc.vector.tensor_add(y[:], y[:], lnb_t[:])
                nc.scalar.dma_start(out_d[r0 : r0 + 128, :], y[:])

            ctx_fin.__exit__(None, None, None)
            ctx_acc.__exit__(None, None, None)
            ctx_sel.__exit__(None, None, None)
            ctx_work.__exit__(None, None, None)
            ctx_gat.__exit__(None, None, None)
            ctx_qps.__exit__(None, None, None)
            ctx_emb.__exit__(None, None, None)
            ctx_idx.__exit__(None, None, None)

    nc.finalize()
    return nc


def _prepare_core_inputs(embeds, edge_index, qTrans, kTrans, vTrans, ln_scale, ln_bias):
    rows = np.asarray(edge_index[0]).astype(np.int64)
    cols = np.asarray(edge_index[1]).astype(np.int64)

    # balanced node -> (bin, slot) assignment: 392 bins of <=128 nodes,
    # degree-balanced so every bin's edge count fits 16 tiles of 128.
    deg = np.bincount(rows, minlength=N)
    order_n = np.argsort(-deg, kind="stable")
    nbins = NCORES * NB
    bin_of = np.empty(N, dtype=np.int64)
    slot_of = np.empty(N, dtype=np.int64)
    loads = np.zeros(nbins, dtype=np.int64)
    cnts = np.zeros(nbins, dtype=np.int64)
    idx = 0
    while idx < N:
        take = min(nbins, N - idx)
        chunk = order_n[idx : idx + take]
        tgt = np.argsort(loads, kind="stable")[:take]
        bin_of[chunk] = tgt
        slot_of[chunk] = cnts[tgt]
        loads[tgt] += deg[chunk]
        cnts[tgt] += 1
        idx += take

    g_all = bin_of[rows]
    order = np.argsort(g_all, kind="stable")
    g = g_all[order]
    cs = cols[order]
    lloc = slot_of[rows[order]].astype(np.float32)

    counts = np.bincount(g, minlength=nbins)
    t_b = max(2, int(np.ceil(counts.max() / 128)))
    cap = t_b * 128

    starts = np.zeros(nbins, dtype=np.int64)
    np.cumsum(counts[:-1], out=starts[1:])
    pos = np.arange(E, dtype=np.int64) - starts[g]
    slot = g * cap + pos

    nslots = nbins * cap
    lloc_a = np.full(nslots, -1.0, dtype=np.float32)
    cidx_a = np.zeros(nslots, dtype=np.int32)
    lloc_a[slot] = lloc
    cidx_a[slot] = cs.astype(np.int32)

    # row layout for the p2 broadcast: [core, NB, cap]
    llocr = lloc_a.reshape(NCORES, NB, cap).astype(ml_dtypes.bfloat16)

    # tile layout: [core, NB*128 partitions, t_b]
    def to_tiles(a):
        a = a.reshape(NCORES, NB, t_b, 128).transpose(0, 1, 3, 2)
        return np.ascontiguousarray(a.reshape(NCORES, NB * 128, t_b))

    lloc_a = to_tiles(lloc_a)
    cidx_a = to_tiles(cidx_a)

    embeds = np.ascontiguousarray(np.asarray(embeds, dtype=np.float32))
    emb_pad = np.zeros((NCORES * NBP, D), dtype=np.float32)
    node_pos = bin_of * 128 + slot_of  # position in [NCORES*NBP] layout
    emb_pad[node_pos] = embeds
    emb_pad = emb_pad.reshape(NCORES, NBP, D)

    qTrans = np.ascontiguousarray(np.asarray(qTrans, dtype=np.float32))
    kTrans = np.ascontiguousarray(np.asarray(kTrans, dtype=np.float32))
    vTrans = np.ascontiguousarray(np.asarray(vTrans, dtype=np.float32))
    ln_scale = np.ascontiguousarray(np.asarray(ln_scale, dtype=np.float32))
    ln_bias = np.ascontiguousarray(np.asarray(ln_bias, dtype=np.float32))

    in_maps = []
    for c in range(NCORES):
        in_maps.append(
            {
                "embeds": embeds,
                "emb_local": emb_pad[c],
                "qT": qTrans,
                "kT": kTrans,
                "vT": vTrans,
                "lnsc": ln_scale,
                "lnb": ln_bias,
                "lloc": lloc_a[c],
                "llocr": llocr[c],
                "cidx": cidx_a[c],
            }
        )
    return in_maps, t_b, node_pos


_PROGRAM_CACHE: dict[int, bass.Bass] = {}


def kernel(embeds, edge_index, qTrans, kTrans, vTrans, ln_scale, ln_bias, **_):
    in_maps, t_b, node_pos = _prepare_core_inputs(
        embeds, edge_index, qTrans, kTrans, vTrans, ln_scale, ln_bias
    )
    nc = _PROGRAM_CACHE.get(t_b)
    if nc is None:
        nc = build_program(t_b)
        _PROGRAM_CACHE[t_b] = nc

    res = run_bass_kernel_spmd(nc, in_maps, core_ids=list(range(NCORES)))
    outs = np.concatenate([res.results[c]["out"] for c in range(NCORES)], axis=0)
    return outs[node_pos]
